# revision 18
# baseline (speedup 1.0000x reference)
"""LorentzLinear forward on 8 Trainium2 NeuronCores.

Computes, for x [65536, 512], W [512, 512], b [512], scale []:
    y      = x @ W.T + b
    time   = sigmoid(y[:, :1]) * exp(scale) + 1.1
    xn     = y[:, 1:]
    denom  = clip(sum(xn * xn, -1, keepdims), 1e-8, None)
    out    = concat([time, xn * sqrt((time^2 - 1) / denom)], -1)

Sharding: data-parallel over rows — 8192 rows per core; W/b/scale replicated.

Device strategy (SPMD, identical program on all cores):
  - The matmul runs in bf16 (fp32 PSUM accumulation). x and W.T are cast to
    bf16 on the host (identical RNE rounding to a device-side cast) so the
    contraction-major x tiles can be loaded with hardware DMA-transpose
    (2-byte dtypes only) — no TensorE transpose pass, and half the input DMA.
  - Per 1024-row block: 4 transposing DMAs produce x.T [128(i), 4(kc), 1024(r)]
    in SBUF; per 128-row tile: 4 accumulating matmuls vs resident W.T.
  - Epilogue per tile: ACT sigmoid on y[:,0]; DVE bn_stats/bn_aggr give
    sum(y^2) = n*(var+mean^2); GpSimd does the small algebra; groups of G=4
    tiles share one batched ACT sqrt so the ACT activation-table set switches
    only twice per group; DVE per-row-scalar multiply writes the scaled
    output (doubling as the PSUM->SBUF copy).
"""

import math

import numpy as np

N, D = 65536, 512
N_CORES = 8
N_PER_CORE = N // N_CORES  # 8192
P = 128
KC = D // P  # 4 contraction chunks
R_BLK = 1024  # rows per DMA-transpose block
N_BLK = N_PER_CORE // R_BLK  # 8
TPB = R_BLK // P  # 8 row tiles per block

# "bf16": single bf16 matmul (rel err ~1e-3)
# "bf16x3": x and W split into hi+lo bf16, 3 matmuls (rel err ~1e-5)
PRECISION = "bf16"

_program_cache = {}


def _build_program(with_bias: bool, precision: str):
    import concourse.bass as bass
    import concourse.tile as tile
    from concourse import bacc, mybir

    FT = mybir.ActivationFunctionType
    ALU = mybir.AluOpType
    f32 = mybir.dt.float32
    bf16 = mybir.dt.bfloat16

    nc = bacc.Bacc(num_devices=N_CORES)
    xb_d = nc.dram_tensor("xb", [N_PER_CORE, D], bf16, kind="ExternalInput")
    wt_d = nc.dram_tensor("wt", [D, D], bf16, kind="ExternalInput")  # W.T [i, o]
    if precision == "bf16x3":
        xlo_d = nc.dram_tensor("xlo", [N_PER_CORE, D], bf16, kind="ExternalInput")
        wtlo_d = nc.dram_tensor("wtlo", [D, D], bf16, kind="ExternalInput")
    es_d = nc.dram_tensor("es", [P, 1], f32, kind="ExternalInput")  # exp(scale)
    if with_bias:
        b_d = nc.dram_tensor("b", [1, D], f32, kind="ExternalInput")
    out_d = nc.dram_tensor("out", [N_PER_CORE, D], f32, kind="ExternalOutput")

    SROWS = 2  # row tiles per output super tile (one store DMA each)
    G = 4  # row tiles per batched epilogue group

    with tile.TileContext(nc) as tc:
        with (
            tc.tile_pool(name="singles", bufs=1) as singles,
            tc.tile_pool(name="xtp", bufs=4) as xtp_pool,
            tc.tile_pool(name="outp", bufs=6) as out_pool,
            tc.tile_pool(name="small", bufs=4) as small,
            tc.tile_pool(name="psum_y", bufs=8, space="PSUM") as psum_y,
        ):
            wt_sb = singles.tile([P, KC, D], bf16)
            nc.sync.dma_start(
                out=wt_sb, in_=wt_d.ap().rearrange("(kc p) o -> p kc o", p=P)
            )
            if precision == "bf16x3":
                wtlo_sb = singles.tile([P, KC, D], bf16)
                nc.sync.dma_start(
                    out=wtlo_sb, in_=wtlo_d.ap().rearrange("(kc p) o -> p kc o", p=P)
                )
            es_sb = singles.tile([P, 1], f32)
            nc.sync.dma_start(out=es_sb, in_=es_d.ap())
            if with_bias:
                b_ap = b_d.ap()
                b_sb = singles.tile([P, D], f32)
                nc.sync.dma_start(
                    out=b_sb,
                    in_=bass.AP(
                        tensor=b_ap.tensor, offset=b_ap.offset, ap=[[0, P], b_ap.ap[1]]
                    ),
                )

            group_y = []  # per row tile in current group: y PSUM tile
            group_out = []  # matching SBUF output slice [P, D]
            group_dmas = []  # (dram_ap, out_sb) flushed at group end
            sg = dg = out_sb = None
            i32 = mybir.dt.int32
            RSQRT_MAGIC = 0x5F3759DF

            for b in range(N_BLK):
                rb = b * R_BLK
                xt_blk = xtp_pool.tile([P, KC, R_BLK], bf16)
                for k in range(KC):
                    nc.sync.dma_start_transpose(
                        xt_blk[:, k, :],
                        xb_d[rb:rb + R_BLK, k * P:(k + 1) * P],
                    )
                if precision == "bf16x3":
                    xtlo_blk = xtp_pool.tile([P, KC, R_BLK], bf16, tag="xtlo")
                    for k in range(KC):
                        nc.sync.dma_start_transpose(
                            xtlo_blk[:, k, :],
                            xlo_d[rb:rb + R_BLK, k * P:(k + 1) * P],
                        )

                for j in range(TPB):
                    ti = b * TPB + j  # global row-tile index
                    gi = ti % G
                    s = ti % SROWS
                    if s == 0:
                        r0 = rb + j * P
                        out_sb = out_pool.tile([P, SROWS, D], f32)
                        group_dmas.append(
                            (
                                out_d[r0:r0 + SROWS * P, :].rearrange(
                                    "(s p) d -> p s d", p=P
                                ),
                                out_sb,
                            )
                        )
                    if gi == 0:
                        sg = small.tile([P, G], f32, tag="sg")  # sigmoid(y0)
                        dg = small.tile([P, G], f32, tag="dg")  # sumsq/(D-1)

                    y_ps = psum_y.tile([P, D], f32)
                    if precision == "bf16x3":
                        mms = [
                            (k, lhs, rhs)
                            for k in range(KC)
                            for lhs, rhs in (
                                (xt_blk, wt_sb),
                                (xt_blk, wtlo_sb),
                                (xtlo_blk, wt_sb),
                            )
                        ]
                    else:
                        mms = [(k, xt_blk, wt_sb) for k in range(KC)]
                    for i_mm, (k, lhs, rhs) in enumerate(mms):
                        nc.tensor.matmul(
                            y_ps,
                            lhsT=lhs[:, k, j * P:(j + 1) * P],
                            rhs=rhs[:, k, :],
                            start=(i_mm == 0),
                            stop=(i_mm == len(mms) - 1),
                        )
                    if with_bias:
                        nc.vector.tensor_add(y_ps, y_ps, b_sb)

                    # Per-tile epilogue inputs. Sigmoid/Copy share ACT table
                    # set 2; the only set switch is the per-group Sqrt.
                    nc.scalar.activation(
                        out=sg[:, gi:gi + 1], in_=y_ps[:, 0:1], func=FT.Sigmoid
                    )
                    # sumsq via bn stats: sum(y^2) = n*(var + mean^2)
                    stats = small.tile([P, 6], f32, tag="stats")
                    nc.vector.bn_stats(out=stats, in_=y_ps[:, 1:])
                    mv = small.tile([P, 2], f32, tag="mv")
                    nc.vector.bn_aggr(out=mv, in_=stats)
                    nc.gpsimd.tensor_scalar(
                        out=dg[:, gi:gi + 1],
                        in0=mv[:, 0:1],
                        scalar1=mv[:, 0:1],
                        scalar2=mv[:, 1:2],
                        op0=ALU.mult,
                        op1=ALU.add,
                    )
                    group_y.append(y_ps)
                    group_out.append(out_sb[:, s, :])

                    if gi == G - 1:
                        # Group epilogue. out[:,1:] = y*sqrt(u/dsum) with
                        # u = t^2-1, dsum = max((D-1)*d', 1e-8), computed as
                        # u*rsqrt(u*dsum) via a quake-seed Newton iteration —
                        # no ACT Sqrt, so the ACT engine never switches
                        # activation-table sets (sigmoid set stays resident).
                        dq = small.tile([P, G], f32, tag="dq")
                        nc.gpsimd.tensor_scalar(
                            out=dq,
                            in0=dg,
                            scalar1=float(D - 1),
                            scalar2=1e-8,
                            op0=ALU.mult,
                            op1=ALU.max,
                        )
                        tg = small.tile([P, G], f32, tag="tg")
                        nc.gpsimd.tensor_scalar(
                            out=tg,
                            in0=sg,
                            scalar1=es_sb,
                            scalar2=1.1,
                            op0=ALU.mult,
                            op1=ALU.add,
                        )
                        ug = small.tile([P, G], f32, tag="ug")
                        nc.gpsimd.tensor_tensor(out=ug, in0=tg, in1=tg, op=ALU.mult)
                        nc.gpsimd.tensor_scalar_add(ug, ug, -1.0)
                        zg = small.tile([P, G], f32, tag="zg")
                        nc.vector.tensor_tensor(out=zg, in0=ug, in1=dq, op=ALU.mult)
                        # rsqrt seed: r = bits_to_f32(MAGIC - (f32_bits(z) >> 1))
                        jt = small.tile([P, G], i32, tag="jt")
                        nc.vector.tensor_scalar(
                            out=jt,
                            in0=zg.bitcast(i32),
                            scalar1=1,
                            scalar2=None,
                            op0=ALU.logical_shift_right,
                        )
                        nc.vector.tensor_scalar(
                            out=jt,
                            in0=jt,
                            scalar1=RSQRT_MAGIC,
                            scalar2=-1,
                            op0=ALU.subtract,
                            op1=ALU.mult,
                        )
                        r = jt.bitcast(f32)
                        n_iters = 3 if precision == "bf16x3" else 2
                        for _ in range(n_iters):
                            ra = small.tile([P, G], f32, tag="ra")
                            nc.vector.tensor_tensor(out=ra, in0=r, in1=r, op=ALU.mult)
                            nc.vector.tensor_tensor(out=ra, in0=ra, in1=zg, op=ALU.mult)
                            nc.vector.tensor_scalar(
                                out=ra,
                                in0=ra,
                                scalar1=-0.5,
                                scalar2=1.5,
                                op0=ALU.mult,
                                op1=ALU.add,
                            )
                            rn = small.tile([P, G], f32, tag="rn")
                            nc.vector.tensor_tensor(out=rn, in0=r, in1=ra, op=ALU.mult)
                            r = rn
                        sqs = small.tile([P, G], f32, tag="sqs")
                        nc.vector.tensor_tensor(out=sqs, in0=ug, in1=r, op=ALU.mult)
                        for i in range(G):
                            nc.scalar.activation(
                                out=group_out[i][:, 1:],
                                in_=group_y[i][:, 1:],
                                func=FT.Copy,
                                scale=sqs[:, i:i + 1],
                            )
                            nc.gpsimd.tensor_copy(
                                out=group_out[i][:, 0:1], in_=tg[:, i:i + 1]
                            )
                        group_y.clear()
                        group_out.clear()
                        # Issue output DMAs from ACT: they directly follow the
                        # finals in the ACT FIFO, so they never block the Sync
                        # FIFO's transpose-DMA issues for upcoming blocks.
                        for dram_ap, sb in group_dmas:
                            nc.scalar.dma_start(out=dram_ap, in_=sb)
                        group_dmas.clear()

            assert not group_y and not group_dmas

    nc.compile()
    return nc


def _get_program(with_bias: bool, precision: str):
    key = (with_bias, precision)
    if key not in _program_cache:
        _program_cache[key] = _build_program(with_bias, precision)
    return _program_cache[key]


TRACE = False
LAST_RESULT = None  # BassKernelResults of the most recent run (for profiling)


def kernel(x, W, b, scale):
    import ml_dtypes
    from concourse.bass_utils import run_bass_kernel_spmd

    global LAST_RESULT

    x = np.asarray(x, dtype=np.float32)
    W = np.asarray(W, dtype=np.float32)
    b = np.asarray(b, dtype=np.float32)
    scale = np.asarray(scale, dtype=np.float32)
    assert x.shape == (N, D) and W.shape == (D, D) and b.shape == (D,)

    with_bias = bool(np.any(b != 0.0))
    nc = _get_program(with_bias, PRECISION)

    xb = x.astype(ml_dtypes.bfloat16)
    wt_f32 = np.ascontiguousarray(W.T)
    wt = wt_f32.astype(ml_dtypes.bfloat16)
    es = np.full((P, 1), np.exp(scale), dtype=np.float32)
    shared = {"wt": wt, "es": es}
    if PRECISION == "bf16x3":
        shared["wtlo"] = (wt_f32 - wt.astype(np.float32)).astype(ml_dtypes.bfloat16)
        xlo = (x - xb.astype(np.float32)).astype(ml_dtypes.bfloat16)
    if with_bias:
        shared["b"] = np.ascontiguousarray(b.reshape(1, D))

    in_maps = []
    for c in range(N_CORES):
        rows = slice(c * N_PER_CORE, (c + 1) * N_PER_CORE)
        m = {"xb": np.ascontiguousarray(xb[rows]), **shared}
        if PRECISION == "bf16x3":
            m["xlo"] = np.ascontiguousarray(xlo[rows])
        in_maps.append(m)
    res = run_bass_kernel_spmd(nc, in_maps, list(range(N_CORES)), trace=TRACE)
    LAST_RESULT = res
    return np.concatenate(
        [res.results[c]["out"] for c in range(N_CORES)], axis=0
    )


# revision 20
# speedup vs baseline: 1.1283x; 1.1283x over previous
"""LorentzLinear forward on 8 Trainium2 NeuronCores.

Computes, for x [65536, 512], W [512, 512], b [512], scale []:
    y      = x @ W.T + b
    time   = sigmoid(y[:, :1]) * exp(scale) + 1.1
    xn     = y[:, 1:]
    denom  = clip(sum(xn * xn, -1, keepdims), 1e-8, None)
    out    = concat([time, xn * sqrt((time^2 - 1) / denom)], -1)

Sharding: data-parallel over rows — 8192 rows per core; W/b/scale replicated.

Device strategy (SPMD, identical program on all cores):
  - The matmul runs in bf16 (fp32 PSUM accumulation). x and W.T are cast to
    bf16 on the host (identical RNE rounding to a device-side cast) so the
    contraction-major x tiles can be loaded with hardware DMA-transpose
    (2-byte dtypes only) — no TensorE transpose pass, and half the input DMA.
  - Per 1024-row block: 4 transposing DMAs produce x.T [128(i), 4(kc), 1024(r)]
    in SBUF; per 128-row tile: 4 accumulating matmuls vs resident W.T.
  - Epilogue per tile: ACT sigmoid on y[:,0]; DVE bn_stats/bn_aggr give
    sum(y^2) = n*(var+mean^2); GpSimd does the small algebra; groups of G=4
    tiles share one batched ACT sqrt so the ACT activation-table set switches
    only twice per group; DVE per-row-scalar multiply writes the scaled
    output (doubling as the PSUM->SBUF copy).
"""

import math

import numpy as np

N, D = 65536, 512
N_CORES = 8
N_PER_CORE = N // N_CORES  # 8192
P = 128
KC = D // P  # 4 contraction chunks
R_BLK = 1024  # rows per DMA-transpose block
N_BLK = N_PER_CORE // R_BLK  # 8
TPB = R_BLK // P  # 8 row tiles per block

# "bf16": single bf16 matmul (rel err ~1e-3)
# "bf16x3": x and W split into hi+lo bf16, 3 matmuls (rel err ~1e-5)
PRECISION = "bf16"

_program_cache = {}


def _build_program(with_bias: bool, precision: str):
    import concourse.bass as bass
    import concourse.tile as tile
    from concourse import bacc, mybir

    FT = mybir.ActivationFunctionType
    ALU = mybir.AluOpType
    f32 = mybir.dt.float32
    bf16 = mybir.dt.bfloat16

    nc = bacc.Bacc(num_devices=N_CORES)
    xb_d = nc.dram_tensor("xb", [N_PER_CORE, D], bf16, kind="ExternalInput")
    wt_d = nc.dram_tensor("wt", [D, D], bf16, kind="ExternalInput")  # W.T [i, o]
    if precision == "bf16x3":
        xlo_d = nc.dram_tensor("xlo", [N_PER_CORE, D], bf16, kind="ExternalInput")
        wtlo_d = nc.dram_tensor("wtlo", [D, D], bf16, kind="ExternalInput")
    es_d = nc.dram_tensor("es", [P, 1], f32, kind="ExternalInput")  # exp(scale)
    if with_bias:
        b_d = nc.dram_tensor("b", [1, D], f32, kind="ExternalInput")
    out_d = nc.dram_tensor("out", [N_PER_CORE, D], f32, kind="ExternalOutput")

    SROWS = 2  # row tiles per output super tile (one store DMA each)
    G = 4  # row tiles per batched epilogue group

    with tile.TileContext(nc) as tc:
        with (
            tc.tile_pool(name="singles", bufs=1) as singles,
            tc.tile_pool(name="xtp", bufs=4) as xtp_pool,
            tc.tile_pool(name="outp", bufs=6) as out_pool,
            tc.tile_pool(name="small", bufs=4) as small,
            tc.tile_pool(name="psum_y", bufs=8, space="PSUM") as psum_y,
        ):
            wt_sb = singles.tile([P, KC, D], bf16)
            nc.sync.dma_start(
                out=wt_sb, in_=wt_d.ap().rearrange("(kc p) o -> p kc o", p=P)
            )
            if precision == "bf16x3":
                wtlo_sb = singles.tile([P, KC, D], bf16)
                nc.sync.dma_start(
                    out=wtlo_sb, in_=wtlo_d.ap().rearrange("(kc p) o -> p kc o", p=P)
                )
            es_sb = singles.tile([P, 1], f32)
            nc.sync.dma_start(out=es_sb, in_=es_d.ap())
            if with_bias:
                b_ap = b_d.ap()
                b_sb = singles.tile([P, D], f32)
                nc.sync.dma_start(
                    out=b_sb,
                    in_=bass.AP(
                        tensor=b_ap.tensor, offset=b_ap.offset, ap=[[0, P], b_ap.ap[1]]
                    ),
                )

            group_y = []  # per row tile in current group: y PSUM tile
            group_out = []  # matching SBUF output slice [P, D]
            group_dmas = []  # (dram_ap, out_sb) flushed at group end
            sg = dg = out_sb = None
            i32 = mybir.dt.int32
            RSQRT_MAGIC = 0x5F3759DF

            def _load_block(b):
                rb = b * R_BLK
                blk = xtp_pool.tile([P, KC, R_BLK], bf16, tag="xt")
                for k in range(KC):
                    nc.sync.dma_start_transpose(
                        blk[:, k, :], xb_d[rb:rb + R_BLK, k * P:(k + 1) * P]
                    )
                if precision == "bf16x3":
                    lo = xtp_pool.tile([P, KC, R_BLK], bf16, tag="xtlo")
                    for k in range(KC):
                        nc.sync.dma_start_transpose(
                            lo[:, k, :], xlo_d[rb:rb + R_BLK, k * P:(k + 1) * P]
                        )
                    return blk, lo
                return blk, None

            next_blk = _load_block(0)
            for b in range(N_BLK):
                rb = b * R_BLK
                xt_blk, xtlo_blk = next_blk
                if b + 1 < N_BLK:
                    # Prefetch: transpose-DMA issues for the next block go into
                    # the Sync FIFO before this block's output DMAs, so a
                    # finals-gated store never delays input supply.
                    next_blk = _load_block(b + 1)

                for j in range(TPB):
                    ti = b * TPB + j  # global row-tile index
                    gi = ti % G
                    s = ti % SROWS
                    if s == 0:
                        r0 = rb + j * P
                        out_sb = out_pool.tile([P, SROWS, D], f32)
                        group_dmas.append(
                            (
                                out_d[r0:r0 + SROWS * P, :].rearrange(
                                    "(s p) d -> p s d", p=P
                                ),
                                out_sb,
                            )
                        )
                    if gi == 0:
                        sg = small.tile([P, G], f32, tag="sg")  # sigmoid(y0)
                        dg = small.tile([P, G], f32, tag="dg")  # sumsq/(D-1)

                    y_ps = psum_y.tile([P, D], f32)
                    if precision == "bf16x3":
                        mms = [
                            (k, lhs, rhs)
                            for k in range(KC)
                            for lhs, rhs in (
                                (xt_blk, wt_sb),
                                (xt_blk, wtlo_sb),
                                (xtlo_blk, wt_sb),
                            )
                        ]
                    else:
                        mms = [(k, xt_blk, wt_sb) for k in range(KC)]
                    for i_mm, (k, lhs, rhs) in enumerate(mms):
                        nc.tensor.matmul(
                            y_ps,
                            lhsT=lhs[:, k, j * P:(j + 1) * P],
                            rhs=rhs[:, k, :],
                            start=(i_mm == 0),
                            stop=(i_mm == len(mms) - 1),
                        )
                    if with_bias:
                        nc.vector.tensor_add(y_ps, y_ps, b_sb)

                    # Per-tile epilogue inputs. Sigmoid/Copy share ACT table
                    # set 2; the only set switch is the per-group Sqrt.
                    nc.scalar.activation(
                        out=sg[:, gi:gi + 1], in_=y_ps[:, 0:1], func=FT.Sigmoid
                    )
                    # sumsq via bn stats: sum(y^2) = n*(var + mean^2)
                    stats = small.tile([P, 6], f32, tag="stats")
                    nc.vector.bn_stats(out=stats, in_=y_ps[:, 1:])
                    mv = small.tile([P, 2], f32, tag="mv")
                    nc.vector.bn_aggr(out=mv, in_=stats)
                    nc.gpsimd.tensor_scalar(
                        out=dg[:, gi:gi + 1],
                        in0=mv[:, 0:1],
                        scalar1=mv[:, 0:1],
                        scalar2=mv[:, 1:2],
                        op0=ALU.mult,
                        op1=ALU.add,
                    )
                    group_y.append(y_ps)
                    group_out.append(out_sb[:, s, :])

                    if gi == G - 1:
                        # Group epilogue. out[:,1:] = y*sqrt(u/dsum) with
                        # u = t^2-1, dsum = max((D-1)*d', 1e-8), computed as
                        # u*rsqrt(u*dsum) via a quake-seed Newton iteration —
                        # no ACT Sqrt, so the ACT engine never switches
                        # activation-table sets (sigmoid set stays resident).
                        dq = small.tile([P, G], f32, tag="dq")
                        nc.gpsimd.tensor_scalar(
                            out=dq,
                            in0=dg,
                            scalar1=float(D - 1),
                            scalar2=1e-8,
                            op0=ALU.mult,
                            op1=ALU.max,
                        )
                        tg = small.tile([P, G], f32, tag="tg")
                        nc.gpsimd.tensor_scalar(
                            out=tg,
                            in0=sg,
                            scalar1=es_sb,
                            scalar2=1.1,
                            op0=ALU.mult,
                            op1=ALU.add,
                        )
                        ug = small.tile([P, G], f32, tag="ug")
                        nc.gpsimd.tensor_tensor(out=ug, in0=tg, in1=tg, op=ALU.mult)
                        nc.gpsimd.tensor_scalar_add(ug, ug, -1.0)
                        zg = small.tile([P, G], f32, tag="zg")
                        nc.vector.tensor_tensor(out=zg, in0=ug, in1=dq, op=ALU.mult)
                        # rsqrt seed: r = bits_to_f32(MAGIC - (f32_bits(z) >> 1))
                        jt = small.tile([P, G], i32, tag="jt")
                        nc.vector.tensor_scalar(
                            out=jt,
                            in0=zg.bitcast(i32),
                            scalar1=1,
                            scalar2=None,
                            op0=ALU.logical_shift_right,
                        )
                        nc.vector.tensor_scalar(
                            out=jt,
                            in0=jt,
                            scalar1=RSQRT_MAGIC,
                            scalar2=-1,
                            op0=ALU.subtract,
                            op1=ALU.mult,
                        )
                        r = jt.bitcast(f32)
                        n_iters = 3 if precision == "bf16x3" else 2
                        for _ in range(n_iters):
                            ra = small.tile([P, G], f32, tag="ra")
                            nc.vector.tensor_tensor(out=ra, in0=r, in1=r, op=ALU.mult)
                            nc.vector.tensor_tensor(out=ra, in0=ra, in1=zg, op=ALU.mult)
                            nc.vector.tensor_scalar(
                                out=ra,
                                in0=ra,
                                scalar1=-0.5,
                                scalar2=1.5,
                                op0=ALU.mult,
                                op1=ALU.add,
                            )
                            rn = small.tile([P, G], f32, tag="rn")
                            nc.vector.tensor_tensor(out=rn, in0=r, in1=ra, op=ALU.mult)
                            r = rn
                        sqs = small.tile([P, G], f32, tag="sqs")
                        nc.vector.tensor_tensor(out=sqs, in0=ug, in1=r, op=ALU.mult)
                        for i in range(G):
                            nc.scalar.activation(
                                out=group_out[i][:, 1:],
                                in_=group_y[i][:, 1:],
                                func=FT.Copy,
                                scale=sqs[:, i:i + 1],
                            )
                            nc.gpsimd.tensor_copy(
                                out=group_out[i][:, 0:1], in_=tg[:, i:i + 1]
                            )
                        group_y.clear()
                        group_out.clear()
                        for dram_ap, sb in group_dmas:
                            nc.sync.dma_start(out=dram_ap, in_=sb)
                        group_dmas.clear()

            assert not group_y and not group_dmas

    nc.compile()
    return nc


def _get_program(with_bias: bool, precision: str):
    key = (with_bias, precision)
    if key not in _program_cache:
        _program_cache[key] = _build_program(with_bias, precision)
    return _program_cache[key]


TRACE = False
LAST_RESULT = None  # BassKernelResults of the most recent run (for profiling)


def kernel(x, W, b, scale):
    import ml_dtypes
    from concourse.bass_utils import run_bass_kernel_spmd

    global LAST_RESULT

    x = np.asarray(x, dtype=np.float32)
    W = np.asarray(W, dtype=np.float32)
    b = np.asarray(b, dtype=np.float32)
    scale = np.asarray(scale, dtype=np.float32)
    assert x.shape == (N, D) and W.shape == (D, D) and b.shape == (D,)

    with_bias = bool(np.any(b != 0.0))
    nc = _get_program(with_bias, PRECISION)

    xb = x.astype(ml_dtypes.bfloat16)
    wt_f32 = np.ascontiguousarray(W.T)
    wt = wt_f32.astype(ml_dtypes.bfloat16)
    es = np.full((P, 1), np.exp(scale), dtype=np.float32)
    shared = {"wt": wt, "es": es}
    if PRECISION == "bf16x3":
        shared["wtlo"] = (wt_f32 - wt.astype(np.float32)).astype(ml_dtypes.bfloat16)
        xlo = (x - xb.astype(np.float32)).astype(ml_dtypes.bfloat16)
    if with_bias:
        shared["b"] = np.ascontiguousarray(b.reshape(1, D))

    in_maps = []
    for c in range(N_CORES):
        rows = slice(c * N_PER_CORE, (c + 1) * N_PER_CORE)
        m = {"xb": np.ascontiguousarray(xb[rows]), **shared}
        if PRECISION == "bf16x3":
            m["xlo"] = np.ascontiguousarray(xlo[rows])
        in_maps.append(m)
    res = run_bass_kernel_spmd(nc, in_maps, list(range(N_CORES)), trace=TRACE)
    LAST_RESULT = res
    return np.concatenate(
        [res.results[c]["out"] for c in range(N_CORES)], axis=0
    )


# revision 22
# speedup vs baseline: 1.2192x; 1.0806x over previous
"""LorentzLinear forward on 8 Trainium2 NeuronCores.

Computes, for x [65536, 512], W [512, 512], b [512], scale []:
    y      = x @ W.T + b
    time   = sigmoid(y[:, :1]) * exp(scale) + 1.1
    xn     = y[:, 1:]
    denom  = clip(sum(xn * xn, -1, keepdims), 1e-8, None)
    out    = concat([time, xn * sqrt((time^2 - 1) / denom)], -1)

Sharding: data-parallel over rows — 8192 rows per core; W/b/scale replicated.

Device strategy (SPMD, identical program on all cores):
  - The matmul runs in bf16 (fp32 PSUM accumulation). x and W.T are cast to
    bf16 on the host (identical RNE rounding to a device-side cast) so the
    contraction-major x tiles can be loaded with hardware DMA-transpose
    (2-byte dtypes only) — no TensorE transpose pass, and half the input DMA.
  - Per 1024-row block: 4 transposing DMAs produce x.T [128(i), 4(kc), 1024(r)]
    in SBUF; per 128-row tile: 4 accumulating matmuls vs resident W.T.
  - Epilogue per tile: ACT sigmoid on y[:,0]; DVE bn_stats/bn_aggr give
    sum(y^2) = n*(var+mean^2); GpSimd does the small algebra; groups of G=4
    tiles share one batched ACT sqrt so the ACT activation-table set switches
    only twice per group; DVE per-row-scalar multiply writes the scaled
    output (doubling as the PSUM->SBUF copy).
"""

import math

import numpy as np

N, D = 65536, 512
N_CORES = 8
N_PER_CORE = N // N_CORES  # 8192
P = 128
KC = D // P  # 4 contraction chunks
R_BLK = 1024  # rows per DMA-transpose block
N_BLK = N_PER_CORE // R_BLK  # 8
TPB = R_BLK // P  # 8 row tiles per block

# "bf16": single bf16 matmul (rel err ~1e-3)
# "bf16x3": x and W split into hi+lo bf16, 3 matmuls (rel err ~1e-5)
PRECISION = "bf16"

_program_cache = {}


def _build_program(with_bias: bool, precision: str):
    import concourse.bass as bass
    import concourse.tile as tile
    from concourse import bacc, mybir

    FT = mybir.ActivationFunctionType
    ALU = mybir.AluOpType
    f32 = mybir.dt.float32
    bf16 = mybir.dt.bfloat16

    nc = bacc.Bacc(num_devices=N_CORES)
    xb_d = nc.dram_tensor("xb", [N_PER_CORE, D], bf16, kind="ExternalInput")
    wt_d = nc.dram_tensor("wt", [D, D], bf16, kind="ExternalInput")  # W.T [i, o]
    if precision == "bf16x3":
        xlo_d = nc.dram_tensor("xlo", [N_PER_CORE, D], bf16, kind="ExternalInput")
        wtlo_d = nc.dram_tensor("wtlo", [D, D], bf16, kind="ExternalInput")
    es_d = nc.dram_tensor("es", [P, 1], f32, kind="ExternalInput")  # exp(scale)
    if with_bias:
        b_d = nc.dram_tensor("b", [1, D], f32, kind="ExternalInput")
    out_d = nc.dram_tensor("out", [N_PER_CORE, D], f32, kind="ExternalOutput")

    SROWS = 2  # row tiles per output super tile (one store DMA each)
    G = 4  # row tiles per batched epilogue group

    with tile.TileContext(nc) as tc:
        with (
            tc.tile_pool(name="singles", bufs=1) as singles,
            tc.tile_pool(name="xtp", bufs=3) as xtp_pool,
            tc.tile_pool(name="outp", bufs=6) as out_pool,
            tc.tile_pool(name="small", bufs=4) as small,
            tc.tile_pool(name="psum_y", bufs=8, space="PSUM") as psum_y,
        ):
            wt_sb = singles.tile([P, KC, D], bf16)
            nc.sync.dma_start(
                out=wt_sb, in_=wt_d.ap().rearrange("(kc p) o -> p kc o", p=P)
            )
            if precision == "bf16x3":
                wtlo_sb = singles.tile([P, KC, D], bf16)
                nc.sync.dma_start(
                    out=wtlo_sb, in_=wtlo_d.ap().rearrange("(kc p) o -> p kc o", p=P)
                )
            es_sb = singles.tile([P, 1], f32)
            nc.sync.dma_start(out=es_sb, in_=es_d.ap())
            if with_bias:
                b_ap = b_d.ap()
                b_sb = singles.tile([P, D], f32)
                nc.sync.dma_start(
                    out=b_sb,
                    in_=bass.AP(
                        tensor=b_ap.tensor, offset=b_ap.offset, ap=[[0, P], b_ap.ap[1]]
                    ),
                )

            group_y = []  # per row tile in current group: y PSUM tile
            group_out = []  # matching SBUF output slice [P, D]
            group_dmas = []  # (dram_ap, out_sb) flushed at group end
            sg = dg = out_sb = None
            i32 = mybir.dt.int32
            RSQRT_MAGIC = 0x5F3759DF

            for b in range(N_BLK):
                rb = b * R_BLK
                xt_blk = xtp_pool.tile([P, KC, R_BLK], bf16, tag="xt")
                for k in range(KC):
                    nc.sync.dma_start_transpose(
                        xt_blk[:, k, :], xb_d[rb:rb + R_BLK, k * P:(k + 1) * P]
                    )
                xtlo_blk = None
                if precision == "bf16x3":
                    xtlo_blk = xtp_pool.tile([P, KC, R_BLK], bf16, tag="xtlo")
                    for k in range(KC):
                        nc.sync.dma_start_transpose(
                            xtlo_blk[:, k, :], xlo_d[rb:rb + R_BLK, k * P:(k + 1) * P]
                        )

                for j in range(TPB):
                    ti = b * TPB + j  # global row-tile index
                    gi = ti % G
                    s = ti % SROWS
                    if s == 0:
                        r0 = rb + j * P
                        out_sb = out_pool.tile([P, SROWS, D], f32)
                        group_dmas.append(
                            (
                                out_d[r0:r0 + SROWS * P, :].rearrange(
                                    "(s p) d -> p s d", p=P
                                ),
                                out_sb,
                            )
                        )
                    if gi == 0:
                        sg = small.tile([P, G], f32, tag="sg")  # sigmoid(y0)
                        dg = small.tile([P, G], f32, tag="dg")  # sumsq/(D-1)

                    y_ps = psum_y.tile([P, D], f32)
                    if precision == "bf16x3":
                        mms = [
                            (k, lhs, rhs)
                            for k in range(KC)
                            for lhs, rhs in (
                                (xt_blk, wt_sb),
                                (xt_blk, wtlo_sb),
                                (xtlo_blk, wt_sb),
                            )
                        ]
                    else:
                        mms = [(k, xt_blk, wt_sb) for k in range(KC)]
                    for i_mm, (k, lhs, rhs) in enumerate(mms):
                        nc.tensor.matmul(
                            y_ps,
                            lhsT=lhs[:, k, j * P:(j + 1) * P],
                            rhs=rhs[:, k, :],
                            start=(i_mm == 0),
                            stop=(i_mm == len(mms) - 1),
                        )
                    if with_bias:
                        nc.vector.tensor_add(y_ps, y_ps, b_sb)

                    # Per-tile epilogue inputs. Sigmoid/Copy share ACT table
                    # set 2; the only set switch is the per-group Sqrt.
                    nc.scalar.activation(
                        out=sg[:, gi:gi + 1], in_=y_ps[:, 0:1], func=FT.Sigmoid
                    )
                    # sumsq via bn stats: sum(y^2) = n*(var + mean^2)
                    stats = small.tile([P, 6], f32, tag="stats")
                    nc.vector.bn_stats(out=stats, in_=y_ps[:, 1:])
                    mv = small.tile([P, 2], f32, tag="mv")
                    nc.vector.bn_aggr(out=mv, in_=stats)
                    nc.gpsimd.tensor_scalar(
                        out=dg[:, gi:gi + 1],
                        in0=mv[:, 0:1],
                        scalar1=mv[:, 0:1],
                        scalar2=mv[:, 1:2],
                        op0=ALU.mult,
                        op1=ALU.add,
                    )
                    group_y.append(y_ps)
                    group_out.append(out_sb[:, s, :])

                    if gi == G - 1:
                        # Group epilogue. out[:,1:] = y*sqrt(u/dsum) with
                        # u = t^2-1, dsum = max((D-1)*d', 1e-8), computed as
                        # u*rsqrt(u*dsum) via a quake-seed Newton iteration —
                        # no ACT Sqrt, so the ACT engine never switches
                        # activation-table sets (sigmoid set stays resident).
                        dq = small.tile([P, G], f32, tag="dq")
                        nc.gpsimd.tensor_scalar(
                            out=dq,
                            in0=dg,
                            scalar1=float(D - 1),
                            scalar2=1e-8,
                            op0=ALU.mult,
                            op1=ALU.max,
                        )
                        tg = small.tile([P, G], f32, tag="tg")
                        nc.gpsimd.tensor_scalar(
                            out=tg,
                            in0=sg,
                            scalar1=es_sb,
                            scalar2=1.1,
                            op0=ALU.mult,
                            op1=ALU.add,
                        )
                        ug = small.tile([P, G], f32, tag="ug")
                        nc.gpsimd.tensor_tensor(out=ug, in0=tg, in1=tg, op=ALU.mult)
                        nc.gpsimd.tensor_scalar_add(ug, ug, -1.0)
                        zg = small.tile([P, G], f32, tag="zg")
                        nc.vector.tensor_tensor(out=zg, in0=ug, in1=dq, op=ALU.mult)
                        # rsqrt seed: r = bits_to_f32(MAGIC - (f32_bits(z) >> 1))
                        jt = small.tile([P, G], i32, tag="jt")
                        nc.vector.tensor_scalar(
                            out=jt,
                            in0=zg.bitcast(i32),
                            scalar1=1,
                            scalar2=None,
                            op0=ALU.logical_shift_right,
                        )
                        nc.vector.tensor_scalar(
                            out=jt,
                            in0=jt,
                            scalar1=RSQRT_MAGIC,
                            scalar2=-1,
                            op0=ALU.subtract,
                            op1=ALU.mult,
                        )
                        r = jt.bitcast(f32)
                        n_iters = 3 if precision == "bf16x3" else 2
                        for _ in range(n_iters):
                            ra = small.tile([P, G], f32, tag="ra")
                            nc.vector.tensor_tensor(out=ra, in0=r, in1=r, op=ALU.mult)
                            nc.vector.tensor_tensor(out=ra, in0=ra, in1=zg, op=ALU.mult)
                            nc.vector.tensor_scalar(
                                out=ra,
                                in0=ra,
                                scalar1=-0.5,
                                scalar2=1.5,
                                op0=ALU.mult,
                                op1=ALU.add,
                            )
                            rn = small.tile([P, G], f32, tag="rn")
                            nc.vector.tensor_tensor(out=rn, in0=r, in1=ra, op=ALU.mult)
                            r = rn
                        sqs = small.tile([P, G], f32, tag="sqs")
                        nc.vector.tensor_tensor(out=sqs, in0=ug, in1=r, op=ALU.mult)
                        for i in range(G):
                            nc.scalar.activation(
                                out=group_out[i][:, 1:],
                                in_=group_y[i][:, 1:],
                                func=FT.Copy,
                                scale=sqs[:, i:i + 1],
                            )
                            nc.gpsimd.tensor_copy(
                                out=group_out[i][:, 0:1], in_=tg[:, i:i + 1]
                            )
                        group_y.clear()
                        group_out.clear()
                        for dram_ap, sb in group_dmas:
                            nc.sync.dma_start(out=dram_ap, in_=sb)
                        group_dmas.clear()

            assert not group_y and not group_dmas

    nc.compile()
    return nc


def _get_program(with_bias: bool, precision: str):
    key = (with_bias, precision)
    if key not in _program_cache:
        _program_cache[key] = _build_program(with_bias, precision)
    return _program_cache[key]


TRACE = False
LAST_RESULT = None  # BassKernelResults of the most recent run (for profiling)


def kernel(x, W, b, scale):
    import ml_dtypes
    from concourse.bass_utils import run_bass_kernel_spmd

    global LAST_RESULT

    x = np.asarray(x, dtype=np.float32)
    W = np.asarray(W, dtype=np.float32)
    b = np.asarray(b, dtype=np.float32)
    scale = np.asarray(scale, dtype=np.float32)
    assert x.shape == (N, D) and W.shape == (D, D) and b.shape == (D,)

    with_bias = bool(np.any(b != 0.0))
    nc = _get_program(with_bias, PRECISION)

    xb = x.astype(ml_dtypes.bfloat16)
    wt_f32 = np.ascontiguousarray(W.T)
    wt = wt_f32.astype(ml_dtypes.bfloat16)
    es = np.full((P, 1), np.exp(scale), dtype=np.float32)
    shared = {"wt": wt, "es": es}
    if PRECISION == "bf16x3":
        shared["wtlo"] = (wt_f32 - wt.astype(np.float32)).astype(ml_dtypes.bfloat16)
        xlo = (x - xb.astype(np.float32)).astype(ml_dtypes.bfloat16)
    if with_bias:
        shared["b"] = np.ascontiguousarray(b.reshape(1, D))

    in_maps = []
    for c in range(N_CORES):
        rows = slice(c * N_PER_CORE, (c + 1) * N_PER_CORE)
        m = {"xb": np.ascontiguousarray(xb[rows]), **shared}
        if PRECISION == "bf16x3":
            m["xlo"] = np.ascontiguousarray(xlo[rows])
        in_maps.append(m)
    res = run_bass_kernel_spmd(nc, in_maps, list(range(N_CORES)), trace=TRACE)
    LAST_RESULT = res
    return np.concatenate(
        [res.results[c]["out"] for c in range(N_CORES)], axis=0
    )


# revision 23
# speedup vs baseline: 1.2256x; 1.0052x over previous
"""LorentzLinear forward on 8 Trainium2 NeuronCores.

Computes, for x [65536, 512], W [512, 512], b [512], scale []:
    y      = x @ W.T + b
    time   = sigmoid(y[:, :1]) * exp(scale) + 1.1
    xn     = y[:, 1:]
    denom  = clip(sum(xn * xn, -1, keepdims), 1e-8, None)
    out    = concat([time, xn * sqrt((time^2 - 1) / denom)], -1)

Sharding: data-parallel over rows — 8192 rows per core; W/b/scale replicated.

Device strategy (SPMD, identical program on all cores):
  - The matmul runs in bf16 (fp32 PSUM accumulation). x and W.T are cast to
    bf16 on the host (identical RNE rounding to a device-side cast) so the
    contraction-major x tiles can be loaded with hardware DMA-transpose
    (2-byte dtypes only) — no TensorE transpose pass, and half the input DMA.
  - Per 1024-row block: 4 transposing DMAs produce x.T [128(i), 4(kc), 1024(r)]
    in SBUF; per 128-row tile: 4 accumulating matmuls vs resident W.T.
  - Epilogue per tile: ACT sigmoid on y[:,0]; DVE bn_stats/bn_aggr give
    sum(y^2) = n*(var+mean^2); GpSimd does the small algebra; groups of G=4
    tiles share one batched ACT sqrt so the ACT activation-table set switches
    only twice per group; DVE per-row-scalar multiply writes the scaled
    output (doubling as the PSUM->SBUF copy).
"""

import math

import numpy as np

N, D = 65536, 512
N_CORES = 8
N_PER_CORE = N // N_CORES  # 8192
P = 128
KC = D // P  # 4 contraction chunks
R_BLK = 1024  # rows per DMA-transpose block
N_BLK = N_PER_CORE // R_BLK  # 8
TPB = R_BLK // P  # 8 row tiles per block

# "bf16": single bf16 matmul (rel err ~1e-3)
# "bf16x3": x and W split into hi+lo bf16, 3 matmuls (rel err ~1e-5)
PRECISION = "bf16"

_program_cache = {}


def _build_program(with_bias: bool, precision: str):
    import concourse.bass as bass
    import concourse.tile as tile
    from concourse import bacc, mybir

    FT = mybir.ActivationFunctionType
    ALU = mybir.AluOpType
    f32 = mybir.dt.float32
    bf16 = mybir.dt.bfloat16

    nc = bacc.Bacc(num_devices=N_CORES)
    xb_d = nc.dram_tensor("xb", [N_PER_CORE, D], bf16, kind="ExternalInput")
    wt_d = nc.dram_tensor("wt", [D, D], bf16, kind="ExternalInput")  # W.T [i, o]
    if precision == "bf16x3":
        xlo_d = nc.dram_tensor("xlo", [N_PER_CORE, D], bf16, kind="ExternalInput")
        wtlo_d = nc.dram_tensor("wtlo", [D, D], bf16, kind="ExternalInput")
    es_d = nc.dram_tensor("es", [P, 1], f32, kind="ExternalInput")  # exp(scale)
    if with_bias:
        b_d = nc.dram_tensor("b", [1, D], f32, kind="ExternalInput")
    out_d = nc.dram_tensor("out", [N_PER_CORE, D], f32, kind="ExternalOutput")

    SROWS = 2  # row tiles per output super tile (one store DMA each)
    G = 4  # row tiles per batched epilogue group

    with tile.TileContext(nc, pool_alloc_mode="queue") as tc:
        with (
            tc.tile_pool(name="singles", bufs=1) as singles,
            tc.tile_pool(name="xtp", bufs=3) as xtp_pool,
            tc.tile_pool(name="outp", bufs=6) as out_pool,
            tc.tile_pool(name="small", bufs=4) as small,
            tc.tile_pool(name="psum_y", bufs=8, space="PSUM") as psum_y,
        ):
            wt_sb = singles.tile([P, KC, D], bf16)
            nc.sync.dma_start(
                out=wt_sb, in_=wt_d.ap().rearrange("(kc p) o -> p kc o", p=P)
            )
            if precision == "bf16x3":
                wtlo_sb = singles.tile([P, KC, D], bf16)
                nc.sync.dma_start(
                    out=wtlo_sb, in_=wtlo_d.ap().rearrange("(kc p) o -> p kc o", p=P)
                )
            es_sb = singles.tile([P, 1], f32)
            nc.sync.dma_start(out=es_sb, in_=es_d.ap())
            if with_bias:
                b_ap = b_d.ap()
                b_sb = singles.tile([P, D], f32)
                nc.sync.dma_start(
                    out=b_sb,
                    in_=bass.AP(
                        tensor=b_ap.tensor, offset=b_ap.offset, ap=[[0, P], b_ap.ap[1]]
                    ),
                )

            group_y = []  # per row tile in current group: y PSUM tile
            group_out = []  # matching SBUF output slice [P, D]
            group_dmas = []  # (dram_ap, out_sb) flushed at group end
            sg = dg = out_sb = None
            i32 = mybir.dt.int32
            RSQRT_MAGIC = 0x5F3759DF

            for b in range(N_BLK):
                rb = b * R_BLK
                xt_blk = xtp_pool.tile([P, KC, R_BLK], bf16, tag="xt")
                for k in range(KC):
                    nc.sync.dma_start_transpose(
                        xt_blk[:, k, :], xb_d[rb:rb + R_BLK, k * P:(k + 1) * P]
                    )
                xtlo_blk = None
                if precision == "bf16x3":
                    xtlo_blk = xtp_pool.tile([P, KC, R_BLK], bf16, tag="xtlo")
                    for k in range(KC):
                        nc.sync.dma_start_transpose(
                            xtlo_blk[:, k, :], xlo_d[rb:rb + R_BLK, k * P:(k + 1) * P]
                        )

                for j in range(TPB):
                    ti = b * TPB + j  # global row-tile index
                    gi = ti % G
                    s = ti % SROWS
                    if s == 0:
                        r0 = rb + j * P
                        out_sb = out_pool.tile([P, SROWS, D], f32)
                        group_dmas.append(
                            (
                                out_d[r0:r0 + SROWS * P, :].rearrange(
                                    "(s p) d -> p s d", p=P
                                ),
                                out_sb,
                            )
                        )
                    if gi == 0:
                        sg = small.tile([P, G], f32, tag="sg")  # sigmoid(y0)
                        dg = small.tile([P, G], f32, tag="dg")  # sumsq/(D-1)

                    y_ps = psum_y.tile([P, D], f32)
                    if precision == "bf16x3":
                        mms = [
                            (k, lhs, rhs)
                            for k in range(KC)
                            for lhs, rhs in (
                                (xt_blk, wt_sb),
                                (xt_blk, wtlo_sb),
                                (xtlo_blk, wt_sb),
                            )
                        ]
                    else:
                        mms = [(k, xt_blk, wt_sb) for k in range(KC)]
                    for i_mm, (k, lhs, rhs) in enumerate(mms):
                        nc.tensor.matmul(
                            y_ps,
                            lhsT=lhs[:, k, j * P:(j + 1) * P],
                            rhs=rhs[:, k, :],
                            start=(i_mm == 0),
                            stop=(i_mm == len(mms) - 1),
                        )
                    if with_bias:
                        nc.vector.tensor_add(y_ps, y_ps, b_sb)

                    # Per-tile epilogue inputs. Sigmoid/Copy share ACT table
                    # set 2; the only set switch is the per-group Sqrt.
                    nc.scalar.activation(
                        out=sg[:, gi:gi + 1], in_=y_ps[:, 0:1], func=FT.Sigmoid
                    )
                    # sumsq via bn stats: sum(y^2) = n*(var + mean^2)
                    stats = small.tile([P, 6], f32, tag="stats")
                    nc.vector.bn_stats(out=stats, in_=y_ps[:, 1:])
                    mv = small.tile([P, 2], f32, tag="mv")
                    nc.vector.bn_aggr(out=mv, in_=stats)
                    nc.gpsimd.tensor_scalar(
                        out=dg[:, gi:gi + 1],
                        in0=mv[:, 0:1],
                        scalar1=mv[:, 0:1],
                        scalar2=mv[:, 1:2],
                        op0=ALU.mult,
                        op1=ALU.add,
                    )
                    group_y.append(y_ps)
                    group_out.append(out_sb[:, s, :])

                    if gi == G - 1:
                        # Group epilogue. out[:,1:] = y*sqrt(u/dsum) with
                        # u = t^2-1, dsum = max((D-1)*d', 1e-8), computed as
                        # u*rsqrt(u*dsum) via a quake-seed Newton iteration —
                        # no ACT Sqrt, so the ACT engine never switches
                        # activation-table sets (sigmoid set stays resident).
                        dq = small.tile([P, G], f32, tag="dq")
                        nc.gpsimd.tensor_scalar(
                            out=dq,
                            in0=dg,
                            scalar1=float(D - 1),
                            scalar2=1e-8,
                            op0=ALU.mult,
                            op1=ALU.max,
                        )
                        tg = small.tile([P, G], f32, tag="tg")
                        nc.gpsimd.tensor_scalar(
                            out=tg,
                            in0=sg,
                            scalar1=es_sb,
                            scalar2=1.1,
                            op0=ALU.mult,
                            op1=ALU.add,
                        )
                        ug = small.tile([P, G], f32, tag="ug")
                        nc.gpsimd.tensor_tensor(out=ug, in0=tg, in1=tg, op=ALU.mult)
                        nc.gpsimd.tensor_scalar_add(ug, ug, -1.0)
                        zg = small.tile([P, G], f32, tag="zg")
                        nc.vector.tensor_tensor(out=zg, in0=ug, in1=dq, op=ALU.mult)
                        # rsqrt seed: r = bits_to_f32(MAGIC - (f32_bits(z) >> 1))
                        jt = small.tile([P, G], i32, tag="jt")
                        nc.vector.tensor_scalar(
                            out=jt,
                            in0=zg.bitcast(i32),
                            scalar1=1,
                            scalar2=None,
                            op0=ALU.logical_shift_right,
                        )
                        nc.vector.tensor_scalar(
                            out=jt,
                            in0=jt,
                            scalar1=RSQRT_MAGIC,
                            scalar2=-1,
                            op0=ALU.subtract,
                            op1=ALU.mult,
                        )
                        r = jt.bitcast(f32)
                        n_iters = 3 if precision == "bf16x3" else 2
                        for _ in range(n_iters):
                            ra = small.tile([P, G], f32, tag="ra")
                            nc.vector.tensor_tensor(out=ra, in0=r, in1=r, op=ALU.mult)
                            nc.vector.tensor_tensor(out=ra, in0=ra, in1=zg, op=ALU.mult)
                            nc.vector.tensor_scalar(
                                out=ra,
                                in0=ra,
                                scalar1=-0.5,
                                scalar2=1.5,
                                op0=ALU.mult,
                                op1=ALU.add,
                            )
                            rn = small.tile([P, G], f32, tag="rn")
                            nc.vector.tensor_tensor(out=rn, in0=r, in1=ra, op=ALU.mult)
                            r = rn
                        sqs = small.tile([P, G], f32, tag="sqs")
                        nc.vector.tensor_tensor(out=sqs, in0=ug, in1=r, op=ALU.mult)
                        for i in range(G):
                            nc.scalar.activation(
                                out=group_out[i][:, 1:],
                                in_=group_y[i][:, 1:],
                                func=FT.Copy,
                                scale=sqs[:, i:i + 1],
                            )
                            nc.gpsimd.tensor_copy(
                                out=group_out[i][:, 0:1], in_=tg[:, i:i + 1]
                            )
                        group_y.clear()
                        group_out.clear()
                        for dram_ap, sb in group_dmas:
                            nc.sync.dma_start(out=dram_ap, in_=sb)
                        group_dmas.clear()

            assert not group_y and not group_dmas

    nc.compile()
    return nc


def _get_program(with_bias: bool, precision: str):
    key = (with_bias, precision)
    if key not in _program_cache:
        _program_cache[key] = _build_program(with_bias, precision)
    return _program_cache[key]


TRACE = False
LAST_RESULT = None  # BassKernelResults of the most recent run (for profiling)


def kernel(x, W, b, scale):
    import ml_dtypes
    from concourse.bass_utils import run_bass_kernel_spmd

    global LAST_RESULT

    x = np.asarray(x, dtype=np.float32)
    W = np.asarray(W, dtype=np.float32)
    b = np.asarray(b, dtype=np.float32)
    scale = np.asarray(scale, dtype=np.float32)
    assert x.shape == (N, D) and W.shape == (D, D) and b.shape == (D,)

    with_bias = bool(np.any(b != 0.0))
    nc = _get_program(with_bias, PRECISION)

    xb = x.astype(ml_dtypes.bfloat16)
    wt_f32 = np.ascontiguousarray(W.T)
    wt = wt_f32.astype(ml_dtypes.bfloat16)
    es = np.full((P, 1), np.exp(scale), dtype=np.float32)
    shared = {"wt": wt, "es": es}
    if PRECISION == "bf16x3":
        shared["wtlo"] = (wt_f32 - wt.astype(np.float32)).astype(ml_dtypes.bfloat16)
        xlo = (x - xb.astype(np.float32)).astype(ml_dtypes.bfloat16)
    if with_bias:
        shared["b"] = np.ascontiguousarray(b.reshape(1, D))

    in_maps = []
    for c in range(N_CORES):
        rows = slice(c * N_PER_CORE, (c + 1) * N_PER_CORE)
        m = {"xb": np.ascontiguousarray(xb[rows]), **shared}
        if PRECISION == "bf16x3":
            m["xlo"] = np.ascontiguousarray(xlo[rows])
        in_maps.append(m)
    res = run_bass_kernel_spmd(nc, in_maps, list(range(N_CORES)), trace=TRACE)
    LAST_RESULT = res
    return np.concatenate(
        [res.results[c]["out"] for c in range(N_CORES)], axis=0
    )
